# revision 21
# baseline (speedup 1.0000x reference)
"""Trainium2 Bass kernel for nn_ConvDiscriminator (ragged CNN discriminator).

Math (per sample b with length L):
  flat = encoder_output[0:L, b, :].ravel()           # contiguous [L*512]
  X[h, l] = flat[h*L + l]  (raw reshape to [512, L], zero-pad cols >= L)
  conv_w (w=1..5): out_w[f, t] = sum_{h,dw} Ww[f,h,dw] * X[h, t+dw]
  pool_w[f] = relu(bias_w[f] + max_{t <= Leff-w} out_w[f, t])
  fc1 -> fc2 -> sigmoid

Kernel strategy (8 cores, uniform SPMD program, per-core data tables):
  - Sort the 128 samples by length desc; slot j holds ranks [8j, 8j+8), one
    per core.  Canonical slot width Ws[j] (mult of 16) = max length in slot,
    baked into the program; per-core lengths live only in data (masks).
  - Slots are bin-packed into "packs" of total width <= 512 so every conv
    matmul streams ~480-512 output columns (the full PSUM bank).  Host
    prepacks, per core, an fp8 tile F[p, k*B + base_s + t] = 16*X_s[4p+k, t]
    (k-tile-major, slots side by side) directly in DRAM -- no on-device
    gather at all, one plain DMA per pack.
  - Conv runs in fp8e4m3 DoubleRow (2x PE throughput): per (pack, w, dw,
    k-pair) one matmul contracting 256 h-rows over all slots of the pack at
    once.  X scaled by 16, W by 512 (exact power-of-2, keeps e4m3 out of
    subnormals); the 1/8192 is folded into the fc1 weights on host.
  - Boundary columns (partial windows at each slot's ragged end + wrap into
    the neighboring slot) are masked with -1e30 via narrow K=1 bf16 matmuls
    (only the inter-core length-spread window needs covering), then
    pool = relu(bias*8192 + reduce_max(psum)).
  - Tiny fc1/fc2/sigmoid on-chip, output [1, 16] per core.
"""

import os
import sys

for _p in ("/opt/trn_rl_repo", "/root/.axon_site/_ro/trn_rl_repo"):
    if os.path.isdir(_p) and _p not in sys.path:
        sys.path.insert(0, _p)

import numpy as np
import ml_dtypes

T = 512
B = 128
H = 512
NF = 128
FS = 5
P = 128
NCORES = 8
NSLOT = B // NCORES  # 16
PACK_CAP = 512  # psum bank columns (f32)
XS = 16.0  # X fp8 scale (power of 2)
WS_SC = 512.0  # W fp8 scale (power of 2)

LAST_EXEC_NS = None
LAST_RESULTS = None
_PROGRAM_CACHE = {}


def _ceil16(x):
    return -(-int(x) // 16) * 16


def _plan(Ws, minL):
    """Derive pack structure + mask geometry from program constants.

    Returns (packs, base, Np, Bp, foff, FTOT, w0, MW, moff, mpoff, MTOT).
    packs: list of slot-id lists; base[j]: column offset of slot j in its
    pack; w0[j]/MW[j]: mask window start (abs col in slot) and width; moff[j]:
    column offset of slot j's 4 mask rows in the msk tensor; mpoff[p]: msk
    column ranges grouped per pack for chunked DMA.
    """
    packs, sums = [], []
    for j in range(NSLOT):
        best, bestslack = -1, None
        for bi in range(len(packs)):
            slack = PACK_CAP - sums[bi] - Ws[j]
            if slack >= 0 and (bestslack is None or slack < bestslack):
                best, bestslack = bi, slack
        if best < 0:
            packs.append([j])
            sums.append(Ws[j])
        else:
            packs[best].append(j)
            sums[best] += Ws[j]

    base = {}
    Np, Bp, foff = [], [], []
    FTOT = 0
    for p, pk in enumerate(packs):
        off = 0
        for j in pk:
            base[j] = off
            off += Ws[j]
        Np.append(off)
        # k-block stride must be 16B-aligned for the DoubleRow rhs pair-step
        Bp.append(_ceil16(off) + 16)
        foff.append(FTOT)
        FTOT += 4 * Bp[-1]

    w0 = [max(0, minL[j] - 4) for j in range(NSLOT)]
    # msk layout (single partition row): [neg 0:128] ++ per (pack, w=2..5) one
    # span block covering psum cols [m0_p, Np) (m0_p = first slot's w0)
    m0 = [w0[pk[0]] for pk in packs]
    moff = {}
    mc = P
    for p, pk in enumerate(packs):
        for w in range(2, FS + 1):
            moff[(p, w)] = mc
            mc += Np[p] - m0[p]
    MTOT = mc
    return packs, base, Np, Bp, foff, FTOT, w0, m0, moff, MTOT


def build_program(Ws, minL):
    import concourse.bass as bass  # noqa: F401
    import concourse.bacc as bacc
    import concourse.mybir as mybir
    from concourse.tile import TileContext

    f32 = mybir.dt.float32
    bf16 = mybir.dt.bfloat16
    f8 = mybir.dt.float8e4
    AX = mybir.AxisListType
    AF = mybir.ActivationFunctionType
    DR = mybir.MatmulPerfMode.DoubleRow

    packs, base, Np, Bp, foff, FTOT, w0, m0, moff, MTOT = _plan(Ws, minL)

    nc = bacc.Bacc()
    fx = [
        nc.declare_dram_parameter(f"fx{p}", [P, 4 * Bp[p]], f8, isOutput=False)
        for p in range(len(packs))
    ]
    msk = nc.declare_dram_parameter("msk", [1, MTOT], bf16, isOutput=False)
    wconv = [
        nc.declare_dram_parameter(f"wc{w}", [P, 4 * w * P], f8, isOutput=False)
        for w in range(1, FS + 1)
    ]
    # cbias*8192 [:, :5] ++ fc1b (col 5) ++ fc2b (col 6 row 0)
    fcon = nc.declare_dram_parameter("fcon", [P, 7], f32, isOutput=False)
    # fc1w/8192 tiles ++ fc2w (col 500) in one bf16 tensor
    fcw = nc.declare_dram_parameter("fcw", [P, 5 * 100 + 1], bf16, isOutput=False)
    out = nc.declare_dram_parameter("out", [1, NSLOT], f32, isOutput=True)

    with TileContext(nc) as tc:
        with (
            tc.tile_pool(name="const", bufs=1) as constp,
            tc.tile_pool(name="pspool", bufs=8, space="PSUM") as pspool,
        ):
            # --- DMAs.  sync: conv weights (per-w tensors) + fc consts;
            # scalar/vector/gpsimd: fx packs round-robin (pack0 split across
            # two queues so compute starts sooner); msk early on gpsimd.
            wsb_t = []
            wsb_t.append(constp.tile([P, 4 * 1 * P], f8, tag="wsb1", name="wsb1"))
            nc.sync.dma_start(out=wsb_t[0][:], in_=wconv[0][:])
            fx_t = []
            ft0 = constp.tile([P, 4 * Bp[0]], f8, tag="fx0", name="fxt0")
            half = 2 * Bp[0]
            nc.scalar.dma_start(out=ft0[:, 0:half], in_=fx[0][:, 0:half])
            nc.gpsimd.dma_start(out=ft0[:, half:], in_=fx[0][:, half:])
            fx_t.append(ft0)
            msk_sb = constp.tile([1, MTOT], bf16, tag="msk")
            nc.gpsimd.dma_start(out=msk_sb[:], in_=msk[:])
            # remaining conv weights early, interleaved across all 3 queues in
            # the order pack0's w-loop consumes them; fx packs follow
            wc_eng = {2: nc.scalar, 3: nc.gpsimd, 4: nc.scalar, 5: nc.sync}
            for w in (2, 3, 5, 4):
                t = constp.tile([P, 4 * w * P], f8, tag=f"wsb{w}", name=f"wsb{w}")
                wc_eng[w].dma_start(out=t[:], in_=wconv[w - 1][:])
                wsb_t.append((w, t))
            wsb_t = [wsb_t[0]] + [t for _, t in sorted(wsb_t[1:], key=lambda x: x[0])]
            fx_eng = [nc.gpsimd, nc.sync, nc.scalar]
            for p in range(1, len(packs)):
                ft = constp.tile([P, 4 * Bp[p]], f8, tag=f"fx{p}", name=f"fxt{p}")
                fx_eng[(p - 1) % 3].dma_start(out=ft[:], in_=fx[p][:])
                fx_t.append(ft)
            fcon_sb = constp.tile([P, 7], f32, tag="fcon")
            nc.sync.dma_start(out=fcon_sb[:], in_=fcon[:])
            cb_sb = fcon_sb[:, 0:FS]
            fc1b_sb = fcon_sb[:100, FS : FS + 1]
            fc2b_sb = fcon_sb[:1, FS + 1 : FS + 2]
            fcw_sb = constp.tile([P, 5 * 100 + 1], bf16, tag="fcw")
            nc.sync.dma_start(out=fcw_sb[:], in_=fcw[:])
            fc1w_sb = fcw_sb[:, 0 : 5 * 100]
            fc2w_sb = fcw_sb[:100, 5 * 100 : 5 * 100 + 1]

            pools = []
            poolsr = []
            for w in range(1, FS + 1):
                pools.append(
                    constp.tile([P, NSLOT], f32, tag=f"pool{w}", name=f"pool{w}")
                )
                poolsr.append(
                    constp.tile([P, NSLOT], bf16, tag=f"poolr{w}", name=f"poolr{w}")
                )

            wsb_r = {
                w: wsb_t[w - 1][:].rearrange("p (b m) -> p b m", m=P)
                for w in range(1, FS + 1)
            }

            for p, pk in enumerate(packs):
                fxr = fx_t[p][:].rearrange("p (k n) -> p k n", k=4)
                N = Np[p]
                for w in range(1, FS + 1):
                    ps = pspool.tile([P, N], f32, tag="ps", name=f"ps{p}w{w}")

                    def conv_mm(dw, q, start):
                        nc.tensor.matmul(
                            ps[:],
                            wsb_r[w][:, dw * 4 + 2 * q : dw * 4 + 2 * q + 2, :],
                            fxr[:, 2 * q : 2 * q + 2, dw : dw + N],
                            start=start,
                            stop=(dw == w - 1 and q == 1),
                            perf_mode=DR,
                        )

                    # start=True must cover the whole psum tile (PSUM
                    # zero-region is bank-granular): full-width conv first,
                    # then one strip-0 mask matmul accumulates -1e30 over the
                    # pack's whole junk span (multi-strip tile_position inside
                    # a DoubleRow group faults the runtime).
                    conv_mm(0, 0, True)
                    if w >= 2:
                        c0 = moff[(p, w)]
                        span = N - m0[p]
                        nc.tensor.matmul(
                            ps[:, m0[p] : N],
                            msk_sb[0:1, 0:P],
                            msk_sb[0:1, c0 : c0 + span],
                            start=False,
                            stop=False,
                            tile_position=(0, 0),
                        )
                    for dw in range(w):
                        for q in (0, 1):
                            if dw == 0 and q == 0:
                                continue
                            conv_mm(dw, q, False)
                    for j in pk:
                        nc.vector.reduce_max(
                            pools[w - 1][:, j : j + 1],
                            ps[:, base[j] : base[j] + Ws[j]],
                            axis=AX.X,
                        )

            # pool_w = relu(8192*(max/8192 + bias)) ; 1/8192 folded into fc1w
            for w in range(1, FS + 1):
                nc.scalar.activation(
                    poolsr[w - 1][:], pools[w - 1][:], AF.Relu, bias=cb_sb[:, w - 1 : w]
                )

            psf1 = pspool.tile([100, NSLOT], f32, tag="ps", name="psf1")
            for k in range(5):
                nc.tensor.matmul(
                    psf1[:],
                    fc1w_sb[:, k * 100 : (k + 1) * 100],
                    poolsr[k][:],
                    start=(k == 0),
                    stop=(k == 4),
                )
            fc1_sb = constp.tile([100, NSLOT], bf16, tag="fc1o")
            nc.scalar.activation(fc1_sb[:], psf1[:], AF.Identity, bias=fc1b_sb)

            psf2 = pspool.tile([1, NSLOT], f32, tag="ps", name="psf2")
            nc.tensor.matmul(psf2[:], fc2w_sb, fc1_sb[:], start=True, stop=True)
            out_sb = constp.tile([1, NSLOT], f32, tag="outsb")
            nc.scalar.activation(out_sb[:], psf2[:], AF.Sigmoid, bias=fc2b_sb)
            nc.sync.dma_start(out=out[:], in_=out_sb[:])

    nc.compile()
    return nc


def _q8(a):
    return np.clip(a, -240.0, 240.0).astype(ml_dtypes.float8_e4m3)


def prepare(encoder_output, lengths, conv_ws, conv_bs, fc1_w, fc1_b, fc2_w, fc2_b):
    """Host-side prep: sample assignment, per-core fp8 prepack, program build."""
    enc = np.ascontiguousarray(np.asarray(encoder_output, dtype=np.float32))
    lens = np.asarray(lengths).astype(np.int64)
    assert enc.shape == (T, B, H)
    assert lens.shape == (B,)

    eff = np.maximum(lens, FS)
    ranks = np.argsort(-eff, kind="stable")
    assignment = [[int(ranks[8 * j + c]) for j in range(NSLOT)] for c in range(NCORES)]
    Ws = tuple(int(eff[ranks[8 * j]]) for j in range(NSLOT))
    minL = tuple(int(eff[ranks[8 * j + 7]]) for j in range(NSLOT))

    packs, base, Np, Bp, foff, FTOT, w0, m0, moff, MTOT = _plan(Ws, minL)
    slot_pack = {}
    for p, pk in enumerate(packs):
        for s, j in enumerate(pk):
            slot_pack[j] = (p, s)

    encT = enc.transpose(1, 0, 2)  # [B, T, H]

    in_maps = []
    for c in range(NCORES):
        fx_c = np.zeros((P, FTOT), dtype=np.float32)
        msk_c = np.zeros((1, MTOT), dtype=np.float32)
        msk_c[0, 0:P] = -1e30
        for j in range(NSLOT):
            b = assignment[c][j]
            L = int(lens[b])
            Le = int(eff[b])
            v = np.zeros((H, Le), dtype=np.float32)
            v[:, :L] = encT[b].reshape(-1)[: H * L].reshape(H, L)
            V4 = (v * XS).reshape(P, 4, Le)
            p, s = slot_pack[j]
            for k in range(4):
                c0 = foff[p] + k * Bp[p] + base[j]
                fx_c[:, c0 : c0 + Le] = V4[:, k, :]
            # junk cols of slot j (abs col t in slot): partial windows at the
            # ragged end + wrap into the next slot's region
            for w in range(2, FS + 1):
                t = np.arange(Ws[j])
                junk = ((t >= Le - w + 1) & (t < Le)) | (t >= Ws[j] - w + 1)
                mc = moff[(p, w)] + base[j] - m0[p]
                sel = np.nonzero(junk)[0]
                msk_c[0, mc + sel] = 1.0
        m = {"msk": msk_c.astype(ml_dtypes.bfloat16)}
        fx8 = _q8(fx_c)
        for p in range(len(packs)):
            m[f"fx{p}"] = np.ascontiguousarray(fx8[:, foff[p] : foff[p] + 4 * Bp[p]])
        in_maps.append(m)

    # weights, shared across cores (per-w tensors; dw-major, then k-tile)
    hsel = np.arange(P)[:, None] * 4
    wtensors = {}
    for w in range(1, FS + 1):
        Ww = np.asarray(conv_ws[w - 1], dtype=np.float32)  # [NF, 1, H, w]
        wc = np.empty((P, 4 * w * P), dtype=np.float32)
        col = 0
        for dw in range(w):
            for k in range(4):
                wc[:, col : col + P] = Ww[:, 0, (hsel + k).ravel(), dw].T * WS_SC
                col += P
        wtensors[f"wc{w}"] = _q8(wc)
    fcon = np.zeros((P, 7), dtype=np.float32)
    fcon[:, 0:FS] = np.stack(
        [np.asarray(bb, dtype=np.float32) * (XS * WS_SC) for bb in conv_bs], axis=1
    )
    fcon[:100, FS] = np.asarray(fc1_b, dtype=np.float32)
    fcon[0, FS + 1] = np.float32(np.asarray(fc2_b, dtype=np.float32).reshape(-1)[0])
    fcw_host = np.zeros((P, 5 * 100 + 1), dtype=np.float32)
    fc1_w = np.asarray(fc1_w, dtype=np.float32)  # [100, 640]
    for k in range(5):
        fcw_host[:, k * 100 : (k + 1) * 100] = fc1_w[:, k * P : (k + 1) * P].T / (XS * WS_SC)
    fcw_host[:100, 5 * 100] = np.asarray(fc2_w, dtype=np.float32).reshape(-1)
    shared = {
        "fcon": fcon,
        "fcw": fcw_host.astype(ml_dtypes.bfloat16),
        **wtensors,
    }
    for m in in_maps:
        m.update(shared)

    key = (Ws, minL)
    if key not in _PROGRAM_CACHE:
        _PROGRAM_CACHE[key] = build_program(Ws, minL)
    nc = _PROGRAM_CACHE[key]
    return nc, in_maps, assignment


def _ensure_ntff_hook():
    """Install the axon NTFF profile hook if the image's antenv lacks it."""
    import types

    try:
        from antenv.axon_hooks import get_axon_ntff_profile_hook  # noqa: F401
        return True
    except ImportError:
        pass
    try:
        import antenv
        from trn_agent_boot.trn_boot import _ntff_profile_via_ctypes

        hook = _ntff_profile_via_ctypes("/opt/axon/libaxon_pjrt.so")
        mod = types.ModuleType("antenv.axon_hooks")
        _state = {"hook": hook}
        mod.get_axon_ntff_profile_hook = lambda: _state["hook"]
        mod.set_axon_ntff_profile_hook = lambda h: _state.update(hook=h)
        sys.modules["antenv.axon_hooks"] = mod
        antenv.axon_hooks = mod
        return hook is not None
    except Exception as e:  # pragma: no cover
        print(f"ntff hook install failed: {e}", file=sys.stderr)
        return False


def kernel(encoder_output, lengths,
           conv_w1, conv_b1, conv_w2, conv_b2, conv_w3, conv_b3,
           conv_w4, conv_b4, conv_w5, conv_b5,
           fc1_w, fc1_b, fc2_w, fc2_b):
    global LAST_EXEC_NS, LAST_RESULTS
    from concourse.bass_utils import run_bass_kernel_spmd

    conv_ws = [conv_w1, conv_w2, conv_w3, conv_w4, conv_w5]
    conv_bs = [conv_b1, conv_b2, conv_b3, conv_b4, conv_b5]
    nc, in_maps, assignment = prepare(
        encoder_output, lengths, conv_ws, conv_bs, fc1_w, fc1_b, fc2_w, fc2_b
    )

    trace = bool(int(os.environ.get("KERNEL_TRACE", "0")))
    if trace:
        trace = _ensure_ntff_hook()
    res = run_bass_kernel_spmd(nc, in_maps, list(range(NCORES)), trace=trace)
    LAST_RESULTS = res
    LAST_EXEC_NS = getattr(res, "exec_time_ns", None)

    out_full = np.empty((B, 1, 1), dtype=np.float32)
    for c in range(NCORES):
        oc = np.asarray(res.results[c]["out"]).reshape(NSLOT)
        for j in range(NSLOT):
            out_full[assignment[c][j], 0, 0] = oc[j]
    return out_full


# revision 25
# speedup vs baseline: 1.0245x; 1.0245x over previous
"""Trainium2 Bass kernel for nn_ConvDiscriminator (ragged CNN discriminator).

Math (per sample b with length L):
  flat = encoder_output[0:L, b, :].ravel()           # contiguous [L*512]
  X[h, l] = flat[h*L + l]  (raw reshape to [512, L], zero-pad cols >= L)
  conv_w (w=1..5): out_w[f, t] = sum_{h,dw} Ww[f,h,dw] * X[h, t+dw]
  pool_w[f] = relu(bias_w[f] + max_{t <= Leff-w} out_w[f, t])
  fc1 -> fc2 -> sigmoid

Kernel strategy (8 cores, uniform SPMD program, per-core data tables):
  - Sort the 128 samples by length desc; slot j holds ranks [8j, 8j+8), one
    per core.  Canonical slot width Ws[j] (mult of 16) = max length in slot,
    baked into the program; per-core lengths live only in data (masks).
  - Slots are bin-packed into "packs" of total width <= 512 so every conv
    matmul streams ~480-512 output columns (the full PSUM bank).  Host
    prepacks, per core, an fp8 tile F[p, k*B + base_s + t] = 16*X_s[4p+k, t]
    (k-tile-major, slots side by side) directly in DRAM -- no on-device
    gather at all, one plain DMA per pack.
  - Conv runs in fp8e4m3 DoubleRow (2x PE throughput): per (pack, w, dw,
    k-pair) one matmul contracting 256 h-rows over all slots of the pack at
    once.  X scaled by 16, W by 512 (exact power-of-2, keeps e4m3 out of
    subnormals); the 1/8192 is folded into the fc1 weights on host.
  - Boundary columns (partial windows at each slot's ragged end + wrap into
    the neighboring slot) are masked with -1e30 via narrow K=1 bf16 matmuls
    (only the inter-core length-spread window needs covering), then
    pool = relu(bias*8192 + reduce_max(psum)).
  - Tiny fc1/fc2/sigmoid on-chip, output [1, 16] per core.
"""

import os
import sys

for _p in ("/opt/trn_rl_repo", "/root/.axon_site/_ro/trn_rl_repo"):
    if os.path.isdir(_p) and _p not in sys.path:
        sys.path.insert(0, _p)

import numpy as np
import ml_dtypes

T = 512
B = 128
H = 512
NF = 128
FS = 5
P = 128
NCORES = 8
NSLOT = B // NCORES  # 16
PACK_CAP = 512  # psum bank columns (f32)
XS = 16.0  # X fp8 scale (power of 2)
WS_SC = 512.0  # W fp8 scale (power of 2)

LAST_EXEC_NS = None
LAST_RESULTS = None
_PROGRAM_CACHE = {}


def _ceil16(x):
    return -(-int(x) // 16) * 16


def _plan(Ws, minL):
    """Derive pack structure + mask geometry from program constants.

    Returns (packs, base, Np, Bp, foff, FTOT, w0, MW, moff, mpoff, MTOT).
    packs: list of slot-id lists; base[j]: column offset of slot j in its
    pack; w0[j]/MW[j]: mask window start (abs col in slot) and width; moff[j]:
    column offset of slot j's 4 mask rows in the msk tensor; mpoff[p]: msk
    column ranges grouped per pack for chunked DMA.
    """
    packs, sums = [], []
    for j in range(NSLOT):
        best, bestslack = -1, None
        for bi in range(len(packs)):
            slack = PACK_CAP - sums[bi] - Ws[j]
            if slack >= 0 and (bestslack is None or slack < bestslack):
                best, bestslack = bi, slack
        if best < 0:
            packs.append([j])
            sums.append(Ws[j])
        else:
            packs[best].append(j)
            sums[best] += Ws[j]

    base = {}
    Np, Bp, foff = [], [], []
    FTOT = 0
    for p, pk in enumerate(packs):
        off = 0
        for j in pk:
            base[j] = off
            off += Ws[j]
        Np.append(off)
        # k-block stride must be 16B-aligned for the DoubleRow rhs pair-step
        Bp.append(_ceil16(off) + 16)
        foff.append(FTOT)
        FTOT += 4 * Bp[-1]

    w0 = [max(0, minL[j] - 4) for j in range(NSLOT)]
    # msk layout (single partition row): [neg 0:128] ++ per (pack, w=2..5) one
    # span block covering psum cols [m0_p, Np) (m0_p = first slot's w0)
    m0 = [w0[pk[0]] for pk in packs]
    moff = {}
    mc = P
    for p, pk in enumerate(packs):
        for w in range(2, FS + 1):
            moff[(p, w)] = mc
            mc += Np[p] - m0[p]
    MTOT = mc
    return packs, base, Np, Bp, foff, FTOT, w0, m0, moff, MTOT


def build_program(Ws, minL):
    import concourse.bass as bass  # noqa: F401
    import concourse.bacc as bacc
    import concourse.mybir as mybir
    from concourse.tile import TileContext

    f32 = mybir.dt.float32
    bf16 = mybir.dt.bfloat16
    f8 = mybir.dt.float8e4
    AX = mybir.AxisListType
    AF = mybir.ActivationFunctionType
    DR = mybir.MatmulPerfMode.DoubleRow

    packs, base, Np, Bp, foff, FTOT, w0, m0, moff, MTOT = _plan(Ws, minL)

    nc = bacc.Bacc()
    fx = [
        nc.declare_dram_parameter(f"fx{p}", [P, 4 * Bp[p]], f8, isOutput=False)
        for p in range(len(packs))
    ]
    msk = nc.declare_dram_parameter("msk", [1, MTOT], bf16, isOutput=False)
    wconv = [
        nc.declare_dram_parameter(f"wc{w}", [P, 4 * w * P], f8, isOutput=False)
        for w in range(1, FS + 1)
    ]
    # cbias*8192 [:, :5] ++ fc1b (col 5) ++ fc2b (col 6 row 0)
    fcon = nc.declare_dram_parameter("fcon", [P, 7], f32, isOutput=False)
    # fc1w/8192 tiles ++ fc2w (col 500) in one bf16 tensor
    fcw = nc.declare_dram_parameter("fcw", [P, 5 * 100 + 1], bf16, isOutput=False)
    out = nc.declare_dram_parameter("out", [1, NSLOT], f32, isOutput=True)

    with TileContext(nc) as tc:
        with (
            tc.tile_pool(name="const", bufs=1) as constp,
            tc.tile_pool(name="pspool", bufs=8, space="PSUM") as pspool,
        ):
            # --- PE pre-warm: dummy matmuls on a zeroed tile keep the PE busy
            # through the ~6us DMA bootstrap so the HAM clock gate reaches
            # 8/8 before real data arrives (else the first ~3.4us of real
            # matmuls run at half clock).
            warm = constp.tile([P, 512], bf16, tag="warm")
            nc.vector.memset(warm[:], 0.0)
            wps = pspool.tile([P, 512], f32, tag="ps", name="warmps")
            for i in range(12):
                nc.tensor.matmul(
                    wps[:], warm[:, 0:P], warm[:],
                    start=(i == 0), stop=(i == 11),
                )

            # --- DMAs.  sync: conv weights (per-w tensors) + fc consts;
            # scalar/vector/gpsimd: fx packs round-robin (pack0 split across
            # two queues so compute starts sooner); msk early on gpsimd.
            wsb_t = []
            wsb_t.append(constp.tile([P, 4 * 1 * P], f8, tag="wsb1", name="wsb1"))
            nc.sync.dma_start(out=wsb_t[0][:], in_=wconv[0][:])
            fx_t = []
            ft0 = constp.tile([P, 4 * Bp[0]], f8, tag="fx0", name="fxt0")
            half = 2 * Bp[0]
            nc.scalar.dma_start(out=ft0[:, 0:half], in_=fx[0][:, 0:half])
            nc.gpsimd.dma_start(out=ft0[:, half:], in_=fx[0][:, half:])
            fx_t.append(ft0)
            msk_sb = constp.tile([1, MTOT], bf16, tag="msk")
            nc.gpsimd.dma_start(out=msk_sb[:], in_=msk[:])
            # remaining conv weights early, split across the fast queues, so
            # pack0's w=2..5 never wait; fx packs follow round-robin
            for w in range(2, FS + 1):
                t = constp.tile([P, 4 * w * P], f8, tag=f"wsb{w}", name=f"wsb{w}")
                eng = nc.scalar if w % 2 == 0 else nc.gpsimd
                eng.dma_start(out=t[:], in_=wconv[w - 1][:])
                wsb_t.append(t)
            fx_eng = [nc.sync, nc.scalar, nc.gpsimd]
            for p in range(1, len(packs)):
                ft = constp.tile([P, 4 * Bp[p]], f8, tag=f"fx{p}", name=f"fxt{p}")
                fx_eng[(p - 1) % 3].dma_start(out=ft[:], in_=fx[p][:])
                fx_t.append(ft)
            fcon_sb = constp.tile([P, 7], f32, tag="fcon")
            nc.sync.dma_start(out=fcon_sb[:], in_=fcon[:])
            cb_sb = fcon_sb[:, 0:FS]
            fc1b_sb = fcon_sb[:100, FS : FS + 1]
            fc2b_sb = fcon_sb[:1, FS + 1 : FS + 2]
            fcw_sb = constp.tile([P, 5 * 100 + 1], bf16, tag="fcw")
            nc.sync.dma_start(out=fcw_sb[:], in_=fcw[:])
            fc1w_sb = fcw_sb[:, 0 : 5 * 100]
            fc2w_sb = fcw_sb[:100, 5 * 100 : 5 * 100 + 1]

            pools = []
            poolsr = []
            for w in range(1, FS + 1):
                pools.append(
                    constp.tile([P, NSLOT], f32, tag=f"pool{w}", name=f"pool{w}")
                )
                poolsr.append(
                    constp.tile([P, NSLOT], bf16, tag=f"poolr{w}", name=f"poolr{w}")
                )

            wsb_r = {
                w: wsb_t[w - 1][:].rearrange("p (b m) -> p b m", m=P)
                for w in range(1, FS + 1)
            }

            for p, pk in enumerate(packs):
                fxr = fx_t[p][:].rearrange("p (k n) -> p k n", k=4)
                N = Np[p]
                for w in range(1, FS + 1):
                    ps = pspool.tile([P, N], f32, tag="ps", name=f"ps{p}w{w}")

                    def conv_mm(dw, q, start):
                        nc.tensor.matmul(
                            ps[:],
                            wsb_r[w][:, dw * 4 + 2 * q : dw * 4 + 2 * q + 2, :],
                            fxr[:, 2 * q : 2 * q + 2, dw : dw + N],
                            start=start,
                            stop=(dw == w - 1 and q == 1),
                            perf_mode=DR,
                        )

                    # start=True must cover the whole psum tile (PSUM
                    # zero-region is bank-granular): full-width conv first,
                    # then one strip-0 mask matmul accumulates -1e30 over the
                    # pack's whole junk span (multi-strip tile_position inside
                    # a DoubleRow group faults the runtime).
                    conv_mm(0, 0, True)
                    if w >= 2:
                        c0 = moff[(p, w)]
                        span = N - m0[p]
                        nc.tensor.matmul(
                            ps[:, m0[p] : N],
                            msk_sb[0:1, 0:P],
                            msk_sb[0:1, c0 : c0 + span],
                            start=False,
                            stop=False,
                            tile_position=(0, 0),
                        )
                    for dw in range(w):
                        for q in (0, 1):
                            if dw == 0 and q == 0:
                                continue
                            conv_mm(dw, q, False)
                    for j in pk:
                        nc.vector.reduce_max(
                            pools[w - 1][:, j : j + 1],
                            ps[:, base[j] : base[j] + Ws[j]],
                            axis=AX.X,
                        )

            # pool_w = relu(8192*(max/8192 + bias)) ; 1/8192 folded into fc1w
            for w in range(1, FS + 1):
                nc.scalar.activation(
                    poolsr[w - 1][:], pools[w - 1][:], AF.Relu, bias=cb_sb[:, w - 1 : w]
                )

            psf1 = pspool.tile([100, NSLOT], f32, tag="ps", name="psf1")
            for k in range(5):
                nc.tensor.matmul(
                    psf1[:],
                    fc1w_sb[:, k * 100 : (k + 1) * 100],
                    poolsr[k][:],
                    start=(k == 0),
                    stop=(k == 4),
                )
            fc1_sb = constp.tile([100, NSLOT], bf16, tag="fc1o")
            nc.scalar.activation(fc1_sb[:], psf1[:], AF.Identity, bias=fc1b_sb)

            psf2 = pspool.tile([1, NSLOT], f32, tag="ps", name="psf2")
            nc.tensor.matmul(psf2[:], fc2w_sb, fc1_sb[:], start=True, stop=True)
            out_sb = constp.tile([1, NSLOT], f32, tag="outsb")
            nc.scalar.activation(out_sb[:], psf2[:], AF.Sigmoid, bias=fc2b_sb)
            # scalar issues the out DMA itself -- no cross-engine sem hop
            nc.scalar.dma_start(out=out[:], in_=out_sb[:])

    nc.compile()
    return nc


def _q8(a):
    return np.clip(a, -240.0, 240.0).astype(ml_dtypes.float8_e4m3)


def prepare(encoder_output, lengths, conv_ws, conv_bs, fc1_w, fc1_b, fc2_w, fc2_b):
    """Host-side prep: sample assignment, per-core fp8 prepack, program build."""
    enc = np.ascontiguousarray(np.asarray(encoder_output, dtype=np.float32))
    lens = np.asarray(lengths).astype(np.int64)
    assert enc.shape == (T, B, H)
    assert lens.shape == (B,)

    eff = np.maximum(lens, FS)
    ranks = np.argsort(-eff, kind="stable")
    assignment = [[int(ranks[8 * j + c]) for j in range(NSLOT)] for c in range(NCORES)]
    Ws = tuple(int(eff[ranks[8 * j]]) for j in range(NSLOT))
    minL = tuple(int(eff[ranks[8 * j + 7]]) for j in range(NSLOT))

    packs, base, Np, Bp, foff, FTOT, w0, m0, moff, MTOT = _plan(Ws, minL)
    slot_pack = {}
    for p, pk in enumerate(packs):
        for s, j in enumerate(pk):
            slot_pack[j] = (p, s)

    encT = enc.transpose(1, 0, 2)  # [B, T, H]

    in_maps = []
    for c in range(NCORES):
        fx_c = np.zeros((P, FTOT), dtype=np.float32)
        msk_c = np.zeros((1, MTOT), dtype=np.float32)
        msk_c[0, 0:P] = -1e30
        for j in range(NSLOT):
            b = assignment[c][j]
            L = int(lens[b])
            Le = int(eff[b])
            v = np.zeros((H, Le), dtype=np.float32)
            v[:, :L] = encT[b].reshape(-1)[: H * L].reshape(H, L)
            V4 = (v * XS).reshape(P, 4, Le)
            p, s = slot_pack[j]
            for k in range(4):
                c0 = foff[p] + k * Bp[p] + base[j]
                fx_c[:, c0 : c0 + Le] = V4[:, k, :]
            # junk cols of slot j (abs col t in slot): partial windows at the
            # ragged end + wrap into the next slot's region
            for w in range(2, FS + 1):
                t = np.arange(Ws[j])
                junk = ((t >= Le - w + 1) & (t < Le)) | (t >= Ws[j] - w + 1)
                mc = moff[(p, w)] + base[j] - m0[p]
                sel = np.nonzero(junk)[0]
                msk_c[0, mc + sel] = 1.0
        m = {"msk": msk_c.astype(ml_dtypes.bfloat16)}
        fx8 = _q8(fx_c)
        for p in range(len(packs)):
            m[f"fx{p}"] = np.ascontiguousarray(fx8[:, foff[p] : foff[p] + 4 * Bp[p]])
        in_maps.append(m)

    # weights, shared across cores (per-w tensors; dw-major, then k-tile)
    hsel = np.arange(P)[:, None] * 4
    wtensors = {}
    for w in range(1, FS + 1):
        Ww = np.asarray(conv_ws[w - 1], dtype=np.float32)  # [NF, 1, H, w]
        wc = np.empty((P, 4 * w * P), dtype=np.float32)
        col = 0
        for dw in range(w):
            for k in range(4):
                wc[:, col : col + P] = Ww[:, 0, (hsel + k).ravel(), dw].T * WS_SC
                col += P
        wtensors[f"wc{w}"] = _q8(wc)
    fcon = np.zeros((P, 7), dtype=np.float32)
    fcon[:, 0:FS] = np.stack(
        [np.asarray(bb, dtype=np.float32) * (XS * WS_SC) for bb in conv_bs], axis=1
    )
    fcon[:100, FS] = np.asarray(fc1_b, dtype=np.float32)
    fcon[0, FS + 1] = np.float32(np.asarray(fc2_b, dtype=np.float32).reshape(-1)[0])
    fcw_host = np.zeros((P, 5 * 100 + 1), dtype=np.float32)
    fc1_w = np.asarray(fc1_w, dtype=np.float32)  # [100, 640]
    for k in range(5):
        fcw_host[:, k * 100 : (k + 1) * 100] = fc1_w[:, k * P : (k + 1) * P].T / (XS * WS_SC)
    fcw_host[:100, 5 * 100] = np.asarray(fc2_w, dtype=np.float32).reshape(-1)
    shared = {
        "fcon": fcon,
        "fcw": fcw_host.astype(ml_dtypes.bfloat16),
        **wtensors,
    }
    for m in in_maps:
        m.update(shared)

    key = (Ws, minL)
    if key not in _PROGRAM_CACHE:
        _PROGRAM_CACHE[key] = build_program(Ws, minL)
    nc = _PROGRAM_CACHE[key]
    return nc, in_maps, assignment


def _ensure_ntff_hook():
    """Install the axon NTFF profile hook if the image's antenv lacks it."""
    import types

    try:
        from antenv.axon_hooks import get_axon_ntff_profile_hook  # noqa: F401
        return True
    except ImportError:
        pass
    try:
        import antenv
        from trn_agent_boot.trn_boot import _ntff_profile_via_ctypes

        hook = _ntff_profile_via_ctypes("/opt/axon/libaxon_pjrt.so")
        mod = types.ModuleType("antenv.axon_hooks")
        _state = {"hook": hook}
        mod.get_axon_ntff_profile_hook = lambda: _state["hook"]
        mod.set_axon_ntff_profile_hook = lambda h: _state.update(hook=h)
        sys.modules["antenv.axon_hooks"] = mod
        antenv.axon_hooks = mod
        return hook is not None
    except Exception as e:  # pragma: no cover
        print(f"ntff hook install failed: {e}", file=sys.stderr)
        return False


def kernel(encoder_output, lengths,
           conv_w1, conv_b1, conv_w2, conv_b2, conv_w3, conv_b3,
           conv_w4, conv_b4, conv_w5, conv_b5,
           fc1_w, fc1_b, fc2_w, fc2_b):
    global LAST_EXEC_NS, LAST_RESULTS
    from concourse.bass_utils import run_bass_kernel_spmd

    conv_ws = [conv_w1, conv_w2, conv_w3, conv_w4, conv_w5]
    conv_bs = [conv_b1, conv_b2, conv_b3, conv_b4, conv_b5]
    nc, in_maps, assignment = prepare(
        encoder_output, lengths, conv_ws, conv_bs, fc1_w, fc1_b, fc2_w, fc2_b
    )

    trace = bool(int(os.environ.get("KERNEL_TRACE", "0")))
    if trace:
        trace = _ensure_ntff_hook()
    res = run_bass_kernel_spmd(nc, in_maps, list(range(NCORES)), trace=trace)
    LAST_RESULTS = res
    LAST_EXEC_NS = getattr(res, "exec_time_ns", None)

    out_full = np.empty((B, 1, 1), dtype=np.float32)
    for c in range(NCORES):
        oc = np.asarray(res.results[c]["out"]).reshape(NSLOT)
        for j in range(NSLOT):
            out_full[assignment[c][j], 0, 0] = oc[j]
    return out_full
